# revision 3
# baseline (speedup 1.0000x reference)
"""HGWaveNet (GraphConv + TCN last-step) Trainium2 kernel, 8 NeuronCores.

Math reduction: with seq = stack([hist0, hist1, hist2, h], axis=2), kernel
size 3, padding (1,1), taking out[:, :, -1] only the last window matters:
    out = hist2 @ W0^T + h @ W1^T + tcn_bias,   Wk = tcn_weight[:, :, k]
    h   = (norm_in * segsum((x*norm_out)[src], dst)) @ gc_w + gc_bias
hist0/hist1 never affect the output.

Sharding: nodes (and their incoming edges) are sharded across 8 cores by dst.
Each core holds a full copy of x in its HBM as the gather table, gathers the
src rows of one 128-edge block per indirect DMA ([128,1] offsets — the only
offset shape that walrus lowers correctly; multi-offset dest rows gather
contiguously on HW), folds the symmetric-norm scale
s_e = rsqrt(max(degout[src],1))*rsqrt(max(degin[dst],1))
into a per-block scaled one-hot S (built on VectorE in one fused
scalar_tensor_tensor op), and aggregates via TensorE:
    aggT[i, n] += G_blk[e, i].T-contracted-with S_blk[e, n]
then out_tile = aggT.T @ (gc_w @ W1^T) + hist2_tile @ W0^T + bias, all on PE.

Degree histograms / edge sorting / layout wrapping are integer graph
preprocessing done on host; all float math runs on device.
"""

import sys

sys.path.insert(0, "/opt/trn_rl_repo")

import numpy as np

import concourse.bass as bass
import concourse.tile as tile
from concourse import bacc, mybir
from concourse.bass_utils import run_bass_kernel_spmd
from concourse.masks import make_identity

F32 = mybir.dt.float32
I32 = mybir.dt.int32

NC_ = 8
TP = 128

LAST_EXEC_NS = None
LAST_RESULT = None

_CACHE = {}


def _build_program(N, D, SH, NT, NB, tile_off, st_size):
    nc = bacc.Bacc(
        "TRN2",
        target_bir_lowering=False,
        debug=False,
        enable_asserts=False,
        num_devices=NC_,
    )

    x_d = nc.dram_tensor("x", [N, D], F32, kind="ExternalInput")
    h2t_d = nc.dram_tensor("h2t", [TP, SH], F32, kind="ExternalInput")
    gsrc_d = nc.dram_tensor("gsrc", [TP, NB], I32, kind="ExternalInput")
    ds_d = nc.dram_tensor("dslot", [TP, NB], F32, kind="ExternalInput")
    da_d = nc.dram_tensor("dega", [TP, NB], F32, kind="ExternalInput")
    db_d = nc.dram_tensor("degb", [TP, NB], F32, kind="ExternalInput")
    gcw_d = nc.dram_tensor("gcw", [D, D], F32, kind="ExternalInput")
    w0_d = nc.dram_tensor("w0", [D, D], F32, kind="ExternalInput")
    w1_d = nc.dram_tensor("w1", [D, D], F32, kind="ExternalInput")
    gcb_d = nc.dram_tensor("gcb", [D, 1], F32, kind="ExternalInput")
    tb_d = nc.dram_tensor("tb", [1, D], F32, kind="ExternalInput")
    iota_d = nc.dram_tensor("iota", [TP, TP], F32, kind="ExternalInput")
    ones_d = nc.dram_tensor("ones", [1, TP], F32, kind="ExternalInput")
    out_d = nc.dram_tensor("out", [SH, D], F32, kind="ExternalOutput")

    from contextlib import ExitStack

    with tile.TileContext(nc) as tc, ExitStack() as ctx:
        cpool = ctx.enter_context(tc.tile_pool(name="const", bufs=1))
        psB = ctx.enter_context(tc.tile_pool(name="psB", bufs=4, space="PSUM"))

        iota_sb = cpool.tile([TP, TP], F32, tag="iota")
        nc.sync.dma_start(iota_sb[:], iota_d[:])
        ident = cpool.tile([TP, TP], F32, tag="ident")
        make_identity(nc, ident[:])

        def load_const(dram, shape, tag):
            t = cpool.tile(shape, F32, tag=tag)
            nc.sync.dma_start(t[:], dram[:])
            return t

        gcw_sb = load_const(gcw_d, [D, D], "gcw")
        w0_sb = load_const(w0_d, [D, D], "w0")
        w1_sb = load_const(w1_d, [D, D], "w1")
        gcb_sb = load_const(gcb_d, [D, 1], "gcb")
        tb_sb = load_const(tb_d, [1, D], "tb")
        ones_sb = load_const(ones_d, [1, TP], "ones")
        h2t_sb = load_const(h2t_d, [TP, SH], "h2t")

        gsrc_sb = cpool.tile([TP, NB], I32, tag="gsrc")
        nc.sync.dma_start(gsrc_sb[:], gsrc_d[:])
        ds_sb = load_const(ds_d, [TP, NB], "dslot")
        da_sb = load_const(da_d, [TP, NB], "dega")
        db_sb = load_const(db_d, [TP, NB], "degb")

        # transposed weights via PE
        def pe_T(src_sb, tag):
            pt = psB.tile([TP, TP], F32, tag="psB")
            nc.tensor.transpose(out=pt[:], in_=src_sb[:], identity=ident[:])
            dst_sb = cpool.tile([TP, TP], F32, tag=tag)
            nc.vector.tensor_copy(dst_sb[:], pt[:])
            return dst_sb

        gcT_sb = pe_T(gcw_sb, "gcT")  # [j, i]
        w0T_sb = pe_T(w0_sb, "w0T")  # [i, o]
        w1T_sb = pe_T(w1_sb, "w1T")  # [j, o]

        # Wc[i, o] = sum_j gc[i, j] * W1[o, j]
        ptc = psB.tile([TP, TP], F32, tag="psB")
        nc.tensor.matmul(out=ptc[:], lhsT=gcT_sb[:], rhs=w1T_sb[:], start=True, stop=True)
        wc_sb = cpool.tile([TP, TP], F32, tag="wc")
        nc.vector.tensor_copy(wc_sb[:], ptc[:])

        # bias_row[o] = sum_j gc_bias[j] * W1[o, j] + tcn_bias[o]
        ptb = psB.tile([TP, TP], F32, tag="psB")
        nc.tensor.matmul(out=ptb[:1, :], lhsT=gcb_sb[:], rhs=w1T_sb[:], start=True, stop=True)
        bias_sb = cpool.tile([1, TP], F32, tag="bias")
        nc.vector.tensor_copy(bias_sb[:], ptb[:1, :])
        nc.vector.tensor_add(bias_sb[:], bias_sb[:], tb_sb[:])

        # per-edge scale s = 1/sqrt(max(da,1)*max(db,1))
        s_sb = cpool.tile([TP, NB], F32, tag="s")
        nc.vector.tensor_scalar_max(db_sb[:], db_sb[:], 1.0)
        nc.vector.scalar_tensor_tensor(
            out=da_sb[:], in0=da_sb[:], scalar=1.0, in1=db_sb[:],
            op0=mybir.AluOpType.max, op1=mybir.AluOpType.mult,
        )
        nc.vector.reciprocal(da_sb[:], da_sb[:])
        nc.scalar.sqrt(s_sb[:], da_sb[:])

        gpool = ctx.enter_context(tc.tile_pool(name="g", bufs=16))
        spool = ctx.enter_context(tc.tile_pool(name="sb1h", bufs=6))
        apool = ctx.enter_context(tc.tile_pool(name="aggt", bufs=4))
        opool = ctx.enter_context(tc.tile_pool(name="osb", bufs=4))
        psA = ctx.enter_context(tc.tile_pool(name="psA", bufs=2, space="PSUM"))

        for st0 in range(0, NT, st_size):
            tiles = list(range(st0, min(st0 + st_size, NT)))
            bb0 = int(tile_off[st0])
            nblk = int(tile_off[tiles[-1] + 1]) - bb0

            psA_t = psA.tile([TP, st_size * TP], F32, tag="psA")
            for j in range(nblk):
                b = bb0 + j
                # which tile does this block belong to
                t = int(np.searchsorted(tile_off, b, side="right")) - 1
                slot = t - st0
                Gb = gpool.tile([TP, D], F32, tag="g")
                nc.gpsimd.indirect_dma_start(
                    out=Gb[:],
                    out_offset=None,
                    in_=x_d[:],
                    in_offset=bass.IndirectOffsetOnAxis(
                        ap=gsrc_sb[:, b:b + 1], axis=0
                    ),
                )
                S_b = spool.tile([TP, TP], F32, tag="sb1h")
                nc.vector.scalar_tensor_tensor(
                    out=S_b[:],
                    in0=iota_sb[:],
                    scalar=ds_sb[:, b:b + 1],
                    in1=s_sb[:, b:b + 1].to_broadcast([TP, TP]),
                    op0=mybir.AluOpType.is_equal,
                    op1=mybir.AluOpType.mult,
                )
                nc.tensor.matmul(
                    out=psA_t[:, slot * TP:(slot + 1) * TP],
                    lhsT=Gb[:],
                    rhs=S_b[:],
                    start=(j == 0),
                    stop=(j == nblk - 1),
                )

            for slot, t in enumerate(tiles):
                nvalid = min(TP, SH - t * TP)
                aggT = apool.tile([TP, TP], F32, tag="aggt")
                nc.scalar.copy(aggT[:], psA_t[:, slot * TP:(slot + 1) * TP])
                pB = psB.tile([TP, TP], F32, tag="psB")
                nc.tensor.matmul(out=pB[:], lhsT=aggT[:], rhs=wc_sb[:], start=True, stop=False)
                nc.tensor.matmul(
                    out=pB[:nvalid, :],
                    lhsT=h2t_sb[:, t * TP:t * TP + nvalid],
                    rhs=w0T_sb[:],
                    start=False,
                    stop=False,
                )
                nc.tensor.matmul(out=pB[:], lhsT=ones_sb[:], rhs=bias_sb[:], start=False, stop=True)
                outt = opool.tile([TP, TP], F32, tag="osb")
                nc.scalar.copy(outt[:], pB[:])
                nc.sync.dma_start(
                    out=out_d[t * TP:t * TP + nvalid, :], in_=outt[:nvalid, :]
                )

    nc.compile()
    return nc


def kernel(**inputs):
    global LAST_EXEC_NS
    x = np.ascontiguousarray(np.asarray(inputs["node_embeddings"], dtype=np.float32))
    gcw = np.ascontiguousarray(np.asarray(inputs["gc_weight"], dtype=np.float32))
    gcb = np.asarray(inputs["gc_bias"], dtype=np.float32)
    tw = np.asarray(inputs["tcn_weight"], dtype=np.float32)
    tb = np.asarray(inputs["tcn_bias"], dtype=np.float32)
    h2 = np.asarray(inputs["hist2"], dtype=np.float32)
    src = np.asarray(inputs["src"]).astype(np.int64)
    dst = np.asarray(inputs["dst"]).astype(np.int64)

    N, D = x.shape
    SH = N // NC_
    NT = (SH + TP - 1) // TP

    # ---- host graph preprocessing (integer only) ----
    deg_out = np.bincount(src, minlength=N)
    deg_in = np.bincount(dst, minlength=N)
    order = np.argsort(dst, kind="stable")
    s_src = src[order]
    s_dst = dst[order]
    core_start = np.searchsorted(s_dst, np.arange(NC_) * SH)
    core_end = np.searchsorted(s_dst, (np.arange(NC_) + 1) * SH)

    cnts = np.zeros((NC_, NT), np.int64)
    locals_ = []
    for c in range(NC_):
        ld = s_dst[core_start[c]:core_end[c]] - c * SH
        locals_.append(ld)
        cnts[c] = np.bincount(ld // TP, minlength=NT)
    Bt = np.maximum(1, -(-cnts.max(axis=0) // TP))  # blocks per tile (global)
    tile_off = np.concatenate([[0], np.cumsum(Bt)]).astype(np.int64)
    NB = int(tile_off[-1])

    gsrc = np.zeros((NC_, TP, NB), np.int32)
    dslot = np.full((NC_, TP, NB), -1.0, np.float32)
    dega = np.ones((NC_, TP, NB), np.float32)
    degb = np.ones((NC_, TP, NB), np.float32)
    for c in range(NC_):
        es = s_src[core_start[c]:core_end[c]]
        ld = locals_[c]
        tl = ld // TP
        tstart = np.searchsorted(tl, np.arange(NT))
        pos = np.arange(len(tl)) - tstart[tl]
        blk = tile_off[tl] + pos // TP
        par = pos % TP
        gsrc[c, par, blk] = es
        dslot[c, par, blk] = (ld % TP).astype(np.float32)
        dega[c, par, blk] = deg_out[es].astype(np.float32)
        degb[c, par, blk] = deg_in[ld + c * SH].astype(np.float32)

    h2t_all = np.ascontiguousarray(h2.T)  # [128, N]
    w0 = np.ascontiguousarray(tw[:, :, 0])
    w1 = np.ascontiguousarray(tw[:, :, 1])
    iota_arr = np.tile(np.arange(TP, dtype=np.float32)[None, :], (TP, 1))
    iota_arr = np.ascontiguousarray(
        np.broadcast_to(np.arange(TP, dtype=np.float32)[None, :], (TP, TP))
    )
    ones_row = np.ones((1, TP), np.float32)

    key = (N, D, SH, NT, NB, tile_off.tobytes())
    if key not in _CACHE:
        _CACHE[key] = _build_program(N, D, SH, NT, NB, tile_off, 4)
    nc = _CACHE[key]

    in_maps = []
    for c in range(NC_):
        in_maps.append(
            {
                "x": x,
                "h2t": np.ascontiguousarray(h2t_all[:, c * SH:(c + 1) * SH]),
                "gsrc": gsrc[c],
                "dslot": dslot[c],
                "dega": dega[c],
                "degb": degb[c],
                "gcw": gcw,
                "w0": w0,
                "w1": w1,
                "gcb": np.ascontiguousarray(gcb.reshape(D, 1)),
                "tb": np.ascontiguousarray(tb.reshape(1, D)),
                "iota": iota_arr,
                "ones": ones_row,
            }
        )

    res = run_bass_kernel_spmd(nc, in_maps, list(range(NC_)))
    LAST_EXEC_NS = res.exec_time_ns
    global LAST_RESULT
    LAST_RESULT = res
    out = np.concatenate([res.results[c]["out"] for c in range(NC_)], axis=0)
    return out

